# revision 33
# baseline (speedup 1.0000x reference)
"""Multi-head attention (B=4, S=2048, D=1024, H=16) on 8 trn2 NeuronCores.

Sharding: core = (batch b, head-group g) with b = core//2, g = core%2.
Each core handles one batch and 8 heads (512 of the 1024 d_model dims).

v2 structure (per core):
  - host pre-relayouts x^T / weights so every device DMA is 128 descriptors
    of contiguous >=4KB runs (kills HWDGE DIRECT2D descriptor-gen cost)
  - QK^T scores are computed in kt PAIRS: kt even uses PE rows 0:64, kt odd
    rows 64:128 (the per-head q/k tiles hold the 64 dims twice), emitted
    interleaved so the two 64-row matmuls stream CONCURRENTLY in separate
    PE row groups
  - softmax exp (ACT) is pipelined one kt-pair deep: iteration i emits
    scores for pair i and PV matmuls for pair i-1, so EXP latency is never
    on the PE critical path
  - PSUM: scores pool 2x[128,1024] (4 banks) + PV accum [65,1024] (2) +
    projection pool 2x[128,512] (2) = all 8 banks; projections never steal
    scores buffers
  - projections (QKV, V, output) are emitted through a fine-grained filler
    queue (~2 matmuls per token) pumped between attention matmuls, with
    readiness counters forcing emission before first use
  - softmax denominator comes out of the PV matmul via a ones-column
    appended to V; normalization (reciprocal + broadcast + mul) runs on
    DVE/gpsimd off the critical path
  - output projection vs Wo[g*512:(g+1)*512, :] gives a partial [2048,1024]
    per core; host sums the two group partials per batch, adds bv@Wo + bo
"""

import os
import numpy as np
from collections import deque
from contextlib import ExitStack

B = 4
S = 2048
D = 1024
H = 16
DK = 64
NCORES = 8
GH = 8           # heads per core (group)
GD = GH * DK     # 512 head dims per core
NCH = GD // 128  # 4 chunks of 128 output dims
KT = S // 128    # 16 key tiles
QC = 1024        # q chunk width for attention
NQC = S // QC    # 2
SC = 512         # s chunk width for projections
NSC = S // SC    # 4
DMT = D // 128   # 8 d_model tiles

MM_DT = os.environ.get("MM_DT", "bf16")  # "bf16" | "f32r"

_CACHE = {}


def _np_mm_dtype():
    if MM_DT == "bf16":
        import ml_dtypes
        return ml_dtypes.bfloat16
    return np.float32


def _build_program():
    import concourse.mybir as mybir
    import concourse.tile as tile
    from concourse import bacc

    f32 = mybir.dt.float32
    dmm = mybir.dt.bfloat16 if MM_DT == "bf16" else mybir.dt.float32r

    nc = bacc.Bacc("TRN2", target_bir_lowering=False, debug=False,
                   num_devices=NCORES)

    # host-relayout inputs: x{q,k,v} as [128, NSC, DMT*SC] where
    # [p, sc, a*SC+s] = x^T[a*128+p, sc*SC+s]  (contiguous 8KB per (p,sc))
    xq = nc.dram_tensor("xq", [128, NSC, DMT * SC], dmm,
                        kind="ExternalInput").ap()
    xk = nc.dram_tensor("xk", [128, NSC, DMT * SC], dmm,
                        kind="ExternalInput").ap()
    xv = nc.dram_tensor("xv", [128, NSC, DMT * SC], dmm,
                        kind="ExternalInput").ap()
    # weights as [128, DMT*GD]: [p, a*GD+d] = W[a*128+p, d]
    wq = nc.dram_tensor("wq", [128, DMT * GD], dmm, kind="ExternalInput").ap()
    wk = nc.dram_tensor("wk", [128, DMT * GD], dmm, kind="ExternalInput").ap()
    wv = nc.dram_tensor("wv", [128, DMT * GD], dmm, kind="ExternalInput").ap()
    # wo as [128, NCH*D]: [p, c*D+n] = Wo[c*128+p, n]
    wo = nc.dram_tensor("wo", [128, NCH * D], dmm, kind="ExternalInput").ap()
    # biases as [128, NCH]: [p, a] = b[a*128+p]
    bq = nc.dram_tensor("bq", [128, NCH], f32, kind="ExternalInput").ap()
    bk = nc.dram_tensor("bk", [128, NCH], f32, kind="ExternalInput").ap()
    out = nc.dram_tensor("out", [S, D], f32, kind="ExternalOutput").ap()

    Exp = mybir.ActivationFunctionType.Exp

    with tile.TileContext(nc) as tc, ExitStack() as ctx:
        # ---- SBUF pools ----
        p_qt = ctx.enter_context(tc.tile_pool(name="qt", bufs=GH))
        p_kt = ctx.enter_context(tc.tile_pool(name="kt", bufs=GH))
        p_v = ctx.enter_context(tc.tile_pool(name="v", bufs=KT))
        p_ot = ctx.enter_context(tc.tile_pool(name="ot", bufs=NCH))
        p_w = ctx.enter_context(tc.tile_pool(name="w", bufs=1))
        p_bias = ctx.enter_context(tc.tile_pool(name="bias", bufs=1))
        p_xs = ctx.enter_context(tc.tile_pool(name="xs", bufs=3))
        p_pt = ctx.enter_context(tc.tile_pool(name="pt", bufs=6))
        p_zr = ctx.enter_context(tc.tile_pool(name="zr", bufs=2))
        p_rb = ctx.enter_context(tc.tile_pool(name="rb", bufs=2))
        p_st = ctx.enter_context(tc.tile_pool(name="st", bufs=2))
        p_ov = ctx.enter_context(tc.tile_pool(name="ov", bufs=3))
        # ---- PSUM: 2*2 + 1*2 + 2*1 = 8 banks ----
        p_ps = ctx.enter_context(tc.tile_pool(name="ps", bufs=2, space="PSUM"))
        p_pv = ctx.enter_context(tc.tile_pool(name="pv", bufs=1, space="PSUM"))
        p_pp = ctx.enter_context(tc.tile_pool(name="pp", bufs=2, space="PSUM"))

        # ---- warm the DMA rings with tiny transfers so the first real
        # DMA doesn't pay ring-init latency ----
        wrmd = p_bias.tile([128, 4], f32, tag="wrmd")
        nc.sync.dma_start(out=wrmd[:, 0:1], in_=bq[:, 0:1])
        nc.scalar.dma_start(out=wrmd[:, 1:2], in_=bq[:, 0:1])
        nc.gpsimd.dma_start(out=wrmd[:, 2:3], in_=bq[:, 0:1])

        # ---- initial DMAs, spread across the 3 DMA-issuing queues ----
        HW = DMT * GD // 2
        wk_sb = p_w.tile([128, DMT * GD], dmm, tag="wk", name="wk_sb")
        nc.scalar.dma_start(out=wk_sb[:, 0:HW], in_=wk[:, 0:HW])
        nc.gpsimd.dma_start(out=wk_sb[:, HW:], in_=wk[:, HW:])
        wq_sb = p_w.tile([128, DMT * GD], dmm, tag="wq", name="wq_sb")
        nc.scalar.dma_start(out=wq_sb[:, 0:HW], in_=wq[:, 0:HW])
        nc.gpsimd.dma_start(out=wq_sb[:, HW:], in_=wq[:, HW:])
        bq_sb = p_bias.tile([128, NCH], f32, tag="bq")
        nc.gpsimd.dma_start(out=bq_sb[:], in_=bq)
        bk_sb = p_bias.tile([128, NCH], f32, tag="bk")
        nc.gpsimd.dma_start(out=bk_sb[:], in_=bk)
        ones_sb = p_bias.tile([128, 1], f32, tag="ones")
        nc.vector.memset(ones_sb[:], 1.0)
        warm = p_bias.tile([128, 512], dmm, tag="warm")
        nc.vector.memset(warm[:], 0.0)
        wps = p_pp.tile([128, SC], f32, tag="pp", name="wps")
        for i in range(4):
            nc.tensor.matmul(out=wps[:], lhsT=warm[:, 0:128], rhs=warm[:],
                             start=(i == 0), stop=(i == 3))

        xs_state = {}
        _dma_rr = [nc.sync, nc.scalar, nc.gpsimd]

        def dma_xs(nm, src, sc, eng, split=False, eng2=None):
            t = p_xs.tile([128, DMT * SC], dmm, tag="xs", name=f"xs_{nm}{sc}")
            if split:
                h = DMT * SC // 2
                eng.dma_start(out=t[:, 0:h], in_=src[:, sc, 0:h])
                (eng2 or eng).dma_start(out=t[:, h:], in_=src[:, sc, h:])
            else:
                eng.dma_start(out=t[:], in_=src[:, sc, :])
            xs_state[(nm, sc)] = t

        # later weight DMAs (needed only once QK c0 chains are done)
        wv_sb = p_w.tile([128, DMT * GD], dmm, tag="wv", name="wv_sb")
        wo_sb = p_w.tile([128, NCH * D], dmm, tag="wo", name="wo_sb")

        qt_sb = [None] * GH
        kt_sb = [None] * GH
        ot_sb = [None] * NCH
        v_sb = [None] * KT

        for hg in range(GH):
            qt_sb[hg] = p_qt.tile([128, S], dmm, tag="qt", name=f"qt{hg}")
            kt_sb[hg] = p_kt.tile([128, S], dmm, tag="kt", name=f"kt{hg}")

        # ---- readiness counters (emission-order correctness) ----
        ready = {"v": 0}           # v_sb tiles emitted
        for c in range(NCH):
            ready[f"k{c}"] = 0     # sc chunks of K^T emitted for pair c
            ready[f"q{c}"] = 0

        fq = deque()

        def pump(budget):
            while fq and budget > 0:
                cost, fn = fq.popleft()
                fn()
                budget -= cost

        def require(key, n):
            while ready[key] < n:
                assert fq, f"filler queue empty but need {key}>={n}"
                cost, fn = fq.popleft()
                fn()

        # ---- projection chain token builders ----
        def qk_chain(nm, c, sc):
            """Tokens computing Q^T/K^T chunk (heads 2c,2c+1, s cols sc*512+)."""
            wsb = wq_sb if nm == "q" else wk_sb
            bsb = bq_sb if nm == "q" else bk_sb
            dsts = ([qt_sb[2 * c], qt_sb[2 * c + 1]] if nm == "q" else
                    [kt_sb[2 * c], kt_sb[2 * c + 1]])
            st = {}

            def mk(a0):
                def f():
                    if a0 == 0:
                        st["pp"] = p_pp.tile([128, SC], f32, tag="pp",
                                             name=f"pp{nm}{c}_{sc}")
                    xs = xs_state[(nm, sc)]
                    for a in (a0, a0 + 1):
                        nc.tensor.matmul(
                            out=st["pp"][:],
                            lhsT=wsb[:, a * GD + c * 128:a * GD + (c + 1) * 128],
                            rhs=xs[:, a * SC:(a + 1) * SC],
                            start=(a == 0), stop=(a == DMT - 1),
                        )
                return f

            def fin():
                ps = st["pp"]
                s0, s1 = sc * SC, (sc + 1) * SC
                # head 2c native rows 0-63; head 2c+1 native rows 64-127
                nc.vector.tensor_scalar_add(
                    out=dsts[0][0:DK, s0:s1], in0=ps[0:DK, :],
                    scalar1=bsb[0:DK, c:c + 1])
                nc.vector.tensor_scalar_add(
                    out=dsts[1][DK:128, s0:s1], in0=ps[DK:128, :],
                    scalar1=bsb[DK:128, c:c + 1])
                # duplicate into the other 64-row half (SBUF->SBUF DMA) so
                # scores can alternate PE row groups
                nc.sync.dma_start(out=dsts[0][DK:128, s0:s1],
                                  in_=dsts[0][0:DK, s0:s1])
                nc.sync.dma_start(out=dsts[1][0:DK, s0:s1],
                                  in_=dsts[1][DK:128, s0:s1])
                ready[f"{nm}{c}"] += 1

            return [(2, mk(0)), (2, mk(2)), (2, mk(4)), (2, mk(6)), (0, fin)]

        def v_chain(st_i):
            """Tokens computing V tile st_i: [128 s, GH, 65] (col 64 = ones)."""
            st = {}

            def mk(a0):
                def f():
                    if a0 == 0:
                        st["pp"] = p_pp.tile([128, GD], f32, tag="pp",
                                             name=f"ppv{st_i}")
                    xv_t = xs_state[("v", st_i // 4)]
                    sub = (st_i % 4) * 128
                    for a in (a0, a0 + 1):
                        nc.tensor.matmul(
                            out=st["pp"][:],
                            lhsT=xv_t[:, a * SC + sub:a * SC + sub + 128],
                            rhs=wv_sb[:, a * GD:(a + 1) * GD],
                            start=(a == 0), stop=(a == DMT - 1),
                        )
                return f

            def fin():
                vt = p_v.tile([128, GH, 65], dmm, tag="v", name=f"v{st_i}")
                nc.vector.tensor_copy(
                    out=vt[:, :, 0:DK],
                    in_=st["pp"][:].rearrange("p (h d) -> p h d", h=GH),
                )
                nc.vector.tensor_copy(
                    out=vt[:, :, DK:65],
                    in_=ones_sb.unsqueeze(1).broadcast_to([128, GH, 1]))
                v_sb[st_i] = vt
                ready["v"] += 1

            return [(2, mk(0)), (2, mk(2)), (2, mk(4)), (2, mk(6)), (0, fin)]

        def fin_chain(qt_i):
            """Output projection rows qt_i*128: out = sum_c ot[c]^T @ Wo[c]."""
            st = {}

            def mk(half, c0):
                def f():
                    if c0 == 0:
                        st[half] = p_pp.tile([128, SC], f32, tag="pp",
                                             name=f"ppf{qt_i}_{half}")
                    for c in (c0, c0 + 1):
                        nc.tensor.matmul(
                            out=st[half][:],
                            lhsT=ot_sb[c][:, qt_i * 128:(qt_i + 1) * 128],
                            rhs=wo_sb[:, c * D + half * 512:
                                      c * D + (half + 1) * 512],
                            start=(c == 0), stop=(c == NCH - 1),
                        )
                return f

            def cp(half):
                def f():
                    if "st" not in st:
                        st["st"] = p_st.tile([128, D], f32, tag="st",
                                             name=f"st{qt_i}")
                    nc.vector.tensor_copy(
                        out=st["st"][:, half * 512:(half + 1) * 512],
                        in_=st[half][:])
                    if half == 1:
                        eng = _dma_rr[qt_i % 3]
                        eng.dma_start(
                            out=out[qt_i * 128:(qt_i + 1) * 128, :],
                            in_=st["st"][:])
                return f

            return [(2, mk(0, 0)), (2, mk(0, 2)), (0, cp(0)),
                    (2, mk(1, 0)), (2, mk(1, 2)), (0, cp(1))]

        # ---- attention pass: heads 2c+hh, q cols [qc*QC, (qc+1)*QC) ----
        def attention_pass(c, qc, hh, ovts, fb=2, ovt_scalar=False,
                           pre=None):
            hg = 2 * c + hh
            require(f"q{c}", 2 * (qc + 1))
            pv_ps = p_pv.tile([65, QC], f32, tag="pv", name=f"pv{c}_{qc}_{hh}")
            pts = {}
            NP = KT // 2
            for ktp in range(NP + 1):
                if ktp == 5 and pre is not None:
                    # emit the NEXT pass's projection prerequisites here,
                    # mid-pass, instead of as a burst at the pass boundary
                    # (which would delay the next scores and starve EXP)
                    pre()
                if ktp < NP:
                    k0, k1 = 2 * ktp, 2 * ktp + 1
                    require(f"k{c}", (k1 * 128) // SC + 1)
                    pump(fb)
                    psA = p_ps.tile([128, QC], f32, tag="ps",
                                    name=f"psA{c}_{qc}_{hh}_{ktp}")
                    psB = p_ps.tile([128, QC], f32, tag="ps",
                                    name=f"psB{c}_{qc}_{hh}_{ktp}")
                    # kt even in PE rows 0:64, kt odd in rows 64:128: the two
                    # halves of one kt share a stationary (one LDWEIGHTS),
                    # and the odd-kt pair can stream in the other row group
                    for half in range(2):
                        q0 = qc * QC + half * 512
                        nc.tensor.matmul(
                            out=psA[:, half * 512:(half + 1) * 512],
                            lhsT=kt_sb[hg][0:DK, k0 * 128:(k0 + 1) * 128],
                            rhs=qt_sb[hg][0:DK, q0:q0 + 512],
                            start=True, stop=True, tile_position=(0, 0),
                        )
                    for half in range(2):
                        q0 = qc * QC + half * 512
                        nc.tensor.matmul(
                            out=psB[:, half * 512:(half + 1) * 512],
                            lhsT=kt_sb[hg][DK:128, k1 * 128:(k1 + 1) * 128],
                            rhs=qt_sb[hg][DK:128, q0:q0 + 512],
                            start=True, stop=True, tile_position=(DK, 0),
                        )
                    ptA = p_pt.tile([128, QC], dmm, tag="pt",
                                    name=f"ptA{c}_{qc}_{hh}_{ktp}")
                    ptB = p_pt.tile([128, QC], dmm, tag="pt",
                                    name=f"ptB{c}_{qc}_{hh}_{ktp}")
                    nc.scalar.activation(ptA[:], psA[:], Exp,
                                         bias=0.0, scale=0.125)
                    nc.scalar.activation(ptB[:], psB[:], Exp,
                                         bias=0.0, scale=0.125)
                    pts[k0], pts[k1] = ptA, ptB
                if ktp >= 1:
                    j0 = 2 * (ktp - 1)
                    require("v", j0 + 2)
                    pump(fb)
                    for j in (j0, j0 + 1):
                        for half in range(2):
                            nc.tensor.matmul(
                                out=pv_ps[:, half * 512:(half + 1) * 512],
                                lhsT=v_sb[j][:, hg, :],
                                rhs=pts[j][:, half * 512:(half + 1) * 512],
                                start=(j == 0), stop=(j == KT - 1),
                            )
            # evict PV psum right away to release its bank pair (NOT on
            # ScalarE: a copy there blocks the strict-FIFO ACT queue on the
            # last PV matmul, starving the next pass's EXPs)
            ovt = p_ov.tile([65, QC], f32, tag="ov", name=f"ov{c}_{qc}_{hh}")
            if ovt_scalar:
                nc.scalar.copy(out=ovt[:], in_=pv_ps[:])
            else:
                nc.vector.tensor_copy(out=ovt[:], in_=pv_ps[:])
            ovts[hh] = ovt

        def normalize_hh(c, qc, hh, ovt, last=False):
            # O^T = PV[0:64] * broadcast(1 / PV[64]); at the tail the sync
            # queue is busy with 512KB output DMAs, so route the tiny
            # reciprocal-scatter DMAs via the (idle) scalar queue there
            eng = nc.scalar if last else nc.sync
            zs = p_zr.tile([DK, QC // DK], f32, tag="zs",
                           name=f"zs{c}_{qc}_{hh}")
            eng.dma_start(out=zs[:], in_=ovt[DK:DK + 1, :])
            nc.vector.reciprocal(out=zs[:], in_=zs[:])
            zr = p_zr.tile([1, QC], f32, tag="zr", name=f"zr{c}_{qc}_{hh}")
            eng.dma_start(out=zr[:], in_=zs[:])

            rb = p_rb.tile([DK, QC], f32, tag="rb",
                           name=f"rb{c}_{qc}_{hh}")
            nc.gpsimd.partition_broadcast(rb[:], zr[:], channels=DK)
            if hh == 0:
                nc.vector.tensor_mul(
                    out=ot_sb[c][0:DK, qc * QC:(qc + 1) * QC],
                    in0=ovt[0:DK, :], in1=rb[:])
            else:
                tmp = p_rb.tile([DK, QC], dmm, tag="rb",
                                name=f"tmp{c}_{qc}")
                nc.vector.tensor_mul(out=tmp[:], in0=ovt[0:DK, :],
                                     in1=rb[:])
                nc.sync.dma_start(
                    out=ot_sb[c][DK:128, qc * QC:(qc + 1) * QC],
                    in_=tmp[:])

        def normalize(c, qc, ovts):
            for hh in range(2):
                normalize_hh(c, qc, hh, ovts[hh])

        # ---- PRE: minimal work to start attention ----
        dma_xs("k", xk, 0, nc.sync, split=True, eng2=nc.scalar)
        dma_xs("q", xq, 0, nc.sync, split=True, eng2=nc.gpsimd)
        dma_xs("q", xq, 1, nc.sync, split=True, eng2=nc.scalar)
        for tok in qk_chain("k", 0, 0):
            tok[1]()
        for tok in qk_chain("q", 0, 0):
            tok[1]()
        nc.gpsimd.dma_start(out=wv_sb[:], in_=wv)
        dma_xs("v", xv, 0, nc.gpsimd)
        for tok in qk_chain("q", 0, 1):
            tok[1]()

        # ---- filler queue: everything else, in consumption order ----
        def push_xs(nm, src, sc, eng):
            fq.append((0, lambda: dma_xs(nm, src, sc, eng)))

        # pair 0 (qc0 passes): V tiles + remaining K c0 chunks + Q c0 qc1
        fq.extend(v_chain(0))
        fq.extend(v_chain(1))
        push_xs("k", xk, 1, nc.sync)
        fq.extend(v_chain(2))
        fq.extend(qk_chain("k", 0, 1))
        fq.extend(v_chain(3))
        push_xs("v", xv, 1, nc.gpsimd)
        push_xs("k", xk, 2, nc.sync)
        fq.extend(v_chain(4))
        fq.extend(qk_chain("k", 0, 2))
        fq.extend(v_chain(5))
        push_xs("k", xk, 3, nc.sync)
        fq.extend(v_chain(6))
        fq.extend(qk_chain("k", 0, 3))
        fq.extend(v_chain(7))
        push_xs("v", xv, 2, nc.gpsimd)
        for st_i in range(8, 12):
            fq.extend(v_chain(st_i))
        push_xs("v", xv, 3, nc.gpsimd)
        for st_i in range(12, 16):
            if st_i == 12:
                push_xs("q", xq, 2, nc.scalar)
            fq.extend(v_chain(st_i))
        fq.append((0, lambda: nc.scalar.dma_start(out=wo_sb[:], in_=wo)))
        fq.extend(qk_chain("q", 0, 2))
        push_xs("q", xq, 3, nc.scalar)
        fq.extend(qk_chain("q", 0, 3))

        # pair 0 qc1 passes: K/Q chains for c=1
        for sc in range(NSC):
            push_xs("k", xk, sc, nc.sync)
            fq.extend(qk_chain("k", 1, sc))
        for sc in range(2):
            push_xs("q", xq, sc, nc.scalar)
            fq.extend(qk_chain("q", 1, sc))
        # pair 1: chains for c=2 (+ q c1 qc1)
        for sc in range(2, NSC):
            push_xs("q", xq, sc, nc.scalar)
            fq.extend(qk_chain("q", 1, sc))
        for sc in range(NSC):
            push_xs("k", xk, sc, nc.sync)
            fq.extend(qk_chain("k", 2, sc))
        for sc in range(2):
            push_xs("q", xq, sc, nc.scalar)
            fq.extend(qk_chain("q", 2, sc))
        # pair 2: chains for c=3 (+ q c2 qc1)
        for sc in range(2, NSC):
            push_xs("q", xq, sc, nc.scalar)
            fq.extend(qk_chain("q", 2, sc))
        for sc in range(NSC):
            push_xs("k", xk, sc, nc.sync)
            fq.extend(qk_chain("k", 3, sc))
        for sc in range(NSC):
            push_xs("q", xq, sc, nc.scalar)
            fq.extend(qk_chain("q", 3, sc))

        # ---- attention pairs ----
        for c in range(NCH):
            ot_sb[c] = p_ot.tile([128, S], dmm, tag="ot", name=f"ot{c}")
            for qc in range(NQC):
                ovts = [None, None]
                if c == NCH - 1 and qc == NQC - 1:
                    # last chunk: hh=1 first so its long normalize chain
                    # (broadcast + mul + SBUF copy) overlaps the hh=0 pass,
                    # leaving only hh=0's short chain for the tail
                    attention_pass(c, qc, 1, ovts, fb=3)
                    normalize_hh(c, qc, 1, ovts[1], last=True)
                    attention_pass(c, qc, 0, ovts, fb=3, ovt_scalar=True)
                    break
                fb = 1 if c == 0 else (2 if c == 1 else 3)
                # next pass's start-requirements, prefetched mid-pass
                if qc + 1 < NQC:
                    cn, qn = c, qc + 1
                else:
                    cn, qn = c + 1, 0

                def mkpre(cn=cn, qn=qn):
                    require(f"q{cn}", 2 * (qn + 1))
                    require(f"k{cn}", 1)
                attention_pass(c, qc, 0, ovts, fb=fb)
                attention_pass(c, qc, 1, ovts, fb=fb, pre=mkpre)
                normalize(c, qc, ovts)
                if c == NCH - 1 and qc == 0:
                    # final projections for q rows 0:1024 become the fillers
                    # of pair 3's qc1 passes
                    for qt_i in range(8):
                        fq.extend(fin_chain(qt_i))
        pump(1 << 30)

        # ---- tail: pipelined final projections for q rows 1024:2048,
        # interleaved with the last pass's normalization so the PE never
        # waits on the (long) reciprocal/broadcast chain ----
        class TailFin:
            def __init__(self, qt_i, use_pp):
                self.qt_i = qt_i
                self.use_pp = use_pp
                self.pool = p_ps
                self.halves = None
                self.whole = None

            def _alloc(self):
                qt_i = self.qt_i
                if self.use_pp:
                    t0 = p_pp.tile([128, SC], f32, tag="pp",
                                   name=f"fpp{qt_i}_0")
                    t1 = p_pp.tile([128, SC], f32, tag="pp",
                                   name=f"fpp{qt_i}_1")
                    self.halves = [t0[:], t1[:]]
                else:
                    tag = "ps" if self.pool is p_ps else "pv"
                    t = self.pool.tile([128, D], f32, tag=tag,
                                       name=f"fps{qt_i}")
                    self.halves = [t[:, 0:512], t[:, 512:1024]]
                    self.whole = t

            def mm(self, c_list):
                if self.halves is None:
                    self._alloc()
                for half in range(2):
                    for cc in c_list:
                        nc.tensor.matmul(
                            out=self.halves[half],
                            lhsT=ot_sb[cc][:, self.qt_i * 128:
                                           (self.qt_i + 1) * 128],
                            rhs=wo_sb[:, cc * D + half * 512:
                                      cc * D + (half + 1) * 512],
                            start=(cc == 0), stop=(cc == NCH - 1),
                        )

            def close(self):
                self.mm([3])
                stt = p_st.tile([128, D], f32, tag="st",
                                name=f"stt{self.qt_i}")
                if self.whole is not None:
                    nc.vector.tensor_copy(out=stt[:], in_=self.whole[:])
                else:
                    for half in range(2):
                        nc.vector.tensor_copy(
                            out=stt[:, half * 512:(half + 1) * 512],
                            in_=self.halves[half])
                _dma_rr[self.qt_i % 3].dma_start(
                    out=out[self.qt_i * 128:(self.qt_i + 1) * 128, :],
                    in_=stt[:])

        fins = {q: TailFin(q, use_pp=(q in (10, 13))) for q in range(8, KT)}
        fins[11].pool = p_pv
        fins[8].mm([0, 1, 2])
        normalize_hh(NCH - 1, NQC - 1, 0, ovts[0], last=True)
        fins[9].mm([0, 1, 2])
        fins[10].mm([0, 1, 2])
        fins[11].mm([0, 1, 2])
        for a, b in ((None, 8), (12, 9), (13, 10), (14, 11), (15, 12),
                     (None, 13), (None, 14), (None, 15)):
            if a is not None:
                fins[a].mm([0, 1, 2])
            fins[b].close()

    nc.compile()
    return nc


def get_program():
    if "nc" not in _CACHE:
        _CACHE["nc"] = _build_program()
    return _CACHE["nc"]


def _relayout_x(xt):
    # x^T [D, S] -> [128, NSC, DMT*SC]: [p, sc, a*SC+s] = xT[a*128+p, sc*SC+s]
    return np.ascontiguousarray(
        xt.reshape(DMT, 128, NSC, SC).transpose(1, 2, 0, 3)
    ).reshape(128, NSC, DMT * SC)


def make_in_maps(inputs):
    dt = _np_mm_dtype()
    q = np.asarray(inputs["query"], np.float32)
    k = np.asarray(inputs["key"], np.float32)
    v = np.asarray(inputs["value"], np.float32)
    Wq = np.asarray(inputs["Wq"], np.float32)
    Wk = np.asarray(inputs["Wk"], np.float32)
    Wv = np.asarray(inputs["Wv"], np.float32)
    Wo = np.asarray(inputs["Wo"], np.float32)
    bq = np.asarray(inputs["bq"], np.float32)
    bk = np.asarray(inputs["bk"], np.float32)
    in_maps = []
    xs_cache = {}
    for b in range(B):
        xs_cache[b] = {
            "xq": _relayout_x(q[b].T.astype(dt)),
            "xk": _relayout_x(k[b].T.astype(dt)),
            "xv": _relayout_x(v[b].T.astype(dt)),
        }
    for core in range(NCORES):
        b, g = core // 2, core % 2
        sl = slice(g * GD, (g + 1) * GD)
        # weights [D, GD] -> [128, DMT*GD]
        wq_a = np.ascontiguousarray(
            Wq[:, sl].astype(dt).reshape(DMT, 128, GD).transpose(1, 0, 2)
        ).reshape(128, DMT * GD)
        wk_a = np.ascontiguousarray(
            Wk[:, sl].astype(dt).reshape(DMT, 128, GD).transpose(1, 0, 2)
        ).reshape(128, DMT * GD)
        wv_a = np.ascontiguousarray(
            Wv[:, sl].astype(dt).reshape(DMT, 128, GD).transpose(1, 0, 2)
        ).reshape(128, DMT * GD)
        # wo [GD, D] -> [128, NCH*D]
        wo_a = np.ascontiguousarray(
            Wo[sl, :].astype(dt).reshape(NCH, 128, D).transpose(1, 0, 2)
        ).reshape(128, NCH * D)
        in_maps.append({
            "xq": xs_cache[b]["xq"],
            "xk": xs_cache[b]["xk"],
            "xv": xs_cache[b]["xv"],
            "wq": wq_a, "wk": wk_a, "wv": wv_a, "wo": wo_a,
            "bq": np.ascontiguousarray(bq[sl].reshape(NCH, 128).T),
            "bk": np.ascontiguousarray(bk[sl].reshape(NCH, 128).T),
        })
    return in_maps


def combine_outputs(results, inputs):
    Wo = np.asarray(inputs["Wo"], np.float32)
    bv = np.asarray(inputs["bv"], np.float32)
    bo = np.asarray(inputs["bo"], np.float32)
    out = np.empty((B, S, D), np.float32)
    for b in range(B):
        out[b] = results[2 * b]["out"] + results[2 * b + 1]["out"]
    out += bv @ Wo + bo
    return out


def kernel(**inputs):
    from concourse.bass_utils import run_bass_kernel_spmd
    nc = get_program()
    in_maps = make_in_maps(inputs)
    res = run_bass_kernel_spmd(nc, in_maps, list(range(NCORES)))
    return combine_outputs(res.results, inputs)


# revision 34
# speedup vs baseline: 1.0056x; 1.0056x over previous
"""Multi-head attention (B=4, S=2048, D=1024, H=16) on 8 trn2 NeuronCores.

Sharding: core = (batch b, head-group g) with b = core//2, g = core%2.
Each core handles one batch and 8 heads (512 of the 1024 d_model dims).

v2 structure (per core):
  - host pre-relayouts x^T / weights so every device DMA is 128 descriptors
    of contiguous >=4KB runs (kills HWDGE DIRECT2D descriptor-gen cost)
  - QK^T scores are computed in kt PAIRS: kt even uses PE rows 0:64, kt odd
    rows 64:128 (the per-head q/k tiles hold the 64 dims twice), emitted
    interleaved so the two 64-row matmuls stream CONCURRENTLY in separate
    PE row groups
  - softmax exp (ACT) is pipelined one kt-pair deep: iteration i emits
    scores for pair i and PV matmuls for pair i-1, so EXP latency is never
    on the PE critical path
  - PSUM: scores pool 2x[128,1024] (4 banks) + PV accum [65,1024] (2) +
    projection pool 2x[128,512] (2) = all 8 banks; projections never steal
    scores buffers
  - projections (QKV, V, output) are emitted through a fine-grained filler
    queue (~2 matmuls per token) pumped between attention matmuls, with
    readiness counters forcing emission before first use
  - softmax denominator comes out of the PV matmul via a ones-column
    appended to V; normalization (reciprocal + broadcast + mul) runs on
    DVE/gpsimd off the critical path
  - output projection vs Wo[g*512:(g+1)*512, :] gives a partial [2048,1024]
    per core; host sums the two group partials per batch, adds bv@Wo + bo
"""

import os
import numpy as np
from collections import deque
from contextlib import ExitStack

B = 4
S = 2048
D = 1024
H = 16
DK = 64
NCORES = 8
GH = 8           # heads per core (group)
GD = GH * DK     # 512 head dims per core
NCH = GD // 128  # 4 chunks of 128 output dims
KT = S // 128    # 16 key tiles
QC = 1024        # q chunk width for attention
NQC = S // QC    # 2
SC = 512         # s chunk width for projections
NSC = S // SC    # 4
DMT = D // 128   # 8 d_model tiles

MM_DT = os.environ.get("MM_DT", "bf16")  # "bf16" | "f32r"

_CACHE = {}


def _np_mm_dtype():
    if MM_DT == "bf16":
        import ml_dtypes
        return ml_dtypes.bfloat16
    return np.float32


def _build_program():
    import concourse.mybir as mybir
    import concourse.tile as tile
    from concourse import bacc

    f32 = mybir.dt.float32
    dmm = mybir.dt.bfloat16 if MM_DT == "bf16" else mybir.dt.float32r

    nc = bacc.Bacc("TRN2", target_bir_lowering=False, debug=False,
                   num_devices=NCORES)

    # host-relayout inputs: x{q,k,v} as [128, NSC, DMT*SC] where
    # [p, sc, a*SC+s] = x^T[a*128+p, sc*SC+s]  (contiguous 8KB per (p,sc))
    xq = nc.dram_tensor("xq", [128, NSC, DMT * SC], dmm,
                        kind="ExternalInput").ap()
    xk = nc.dram_tensor("xk", [128, NSC, DMT * SC], dmm,
                        kind="ExternalInput").ap()
    xv = nc.dram_tensor("xv", [128, NSC, DMT * SC], dmm,
                        kind="ExternalInput").ap()
    # weights as [128, DMT*GD]: [p, a*GD+d] = W[a*128+p, d]
    wq = nc.dram_tensor("wq", [128, DMT * GD], dmm, kind="ExternalInput").ap()
    wk = nc.dram_tensor("wk", [128, DMT * GD], dmm, kind="ExternalInput").ap()
    wv = nc.dram_tensor("wv", [128, DMT * GD], dmm, kind="ExternalInput").ap()
    # wo as [128, NCH*D]: [p, c*D+n] = Wo[c*128+p, n]
    wo = nc.dram_tensor("wo", [128, NCH * D], dmm, kind="ExternalInput").ap()
    # biases as [128, NCH]: [p, a] = b[a*128+p]
    bq = nc.dram_tensor("bq", [128, NCH], f32, kind="ExternalInput").ap()
    bk = nc.dram_tensor("bk", [128, NCH], f32, kind="ExternalInput").ap()
    out = nc.dram_tensor("out", [S, D], f32, kind="ExternalOutput").ap()

    Exp = mybir.ActivationFunctionType.Exp

    with tile.TileContext(nc) as tc, ExitStack() as ctx:
        # ---- SBUF pools ----
        p_qt = ctx.enter_context(tc.tile_pool(name="qt", bufs=GH))
        p_kt = ctx.enter_context(tc.tile_pool(name="kt", bufs=GH))
        p_v = ctx.enter_context(tc.tile_pool(name="v", bufs=KT))
        p_ot = ctx.enter_context(tc.tile_pool(name="ot", bufs=NCH))
        p_w = ctx.enter_context(tc.tile_pool(name="w", bufs=1))
        p_bias = ctx.enter_context(tc.tile_pool(name="bias", bufs=1))
        p_xs = ctx.enter_context(tc.tile_pool(name="xs", bufs=3))
        p_pt = ctx.enter_context(tc.tile_pool(name="pt", bufs=6))
        p_zr = ctx.enter_context(tc.tile_pool(name="zr", bufs=2))
        p_rb = ctx.enter_context(tc.tile_pool(name="rb", bufs=2))
        p_st = ctx.enter_context(tc.tile_pool(name="st", bufs=2))
        p_ov = ctx.enter_context(tc.tile_pool(name="ov", bufs=3))
        # ---- PSUM: 2*2 + 1*2 + 2*1 = 8 banks ----
        p_ps = ctx.enter_context(tc.tile_pool(name="ps", bufs=2, space="PSUM"))
        p_pv = ctx.enter_context(tc.tile_pool(name="pv", bufs=1, space="PSUM"))
        p_pp = ctx.enter_context(tc.tile_pool(name="pp", bufs=2, space="PSUM"))

        # ---- warm the DMA rings with tiny transfers so the first real
        # DMA doesn't pay ring-init latency ----
        wrmd = p_bias.tile([128, 4], f32, tag="wrmd")
        nc.sync.dma_start(out=wrmd[:, 0:1], in_=bq[:, 0:1])
        nc.scalar.dma_start(out=wrmd[:, 1:2], in_=bq[:, 0:1])
        nc.gpsimd.dma_start(out=wrmd[:, 2:3], in_=bq[:, 0:1])

        # ---- initial DMAs, spread across the 3 DMA-issuing queues ----
        HW = DMT * GD // 2
        wk_sb = p_w.tile([128, DMT * GD], dmm, tag="wk", name="wk_sb")
        nc.scalar.dma_start(out=wk_sb[:, 0:HW], in_=wk[:, 0:HW])
        nc.gpsimd.dma_start(out=wk_sb[:, HW:], in_=wk[:, HW:])
        wq_sb = p_w.tile([128, DMT * GD], dmm, tag="wq", name="wq_sb")
        nc.scalar.dma_start(out=wq_sb[:, 0:HW], in_=wq[:, 0:HW])
        nc.gpsimd.dma_start(out=wq_sb[:, HW:], in_=wq[:, HW:])
        bq_sb = p_bias.tile([128, NCH], f32, tag="bq")
        nc.gpsimd.dma_start(out=bq_sb[:], in_=bq)
        bk_sb = p_bias.tile([128, NCH], f32, tag="bk")
        nc.gpsimd.dma_start(out=bk_sb[:], in_=bk)
        ones_sb = p_bias.tile([128, 1], f32, tag="ones")
        nc.vector.memset(ones_sb[:], 1.0)
        warm = p_bias.tile([128, 512], dmm, tag="warm")
        nc.vector.memset(warm[:], 0.0)
        wps = p_pp.tile([128, SC], f32, tag="pp", name="wps")
        for i in range(4):
            nc.tensor.matmul(out=wps[:], lhsT=warm[:, 0:128], rhs=warm[:],
                             start=(i == 0), stop=(i == 3))

        xs_state = {}
        _dma_rr = [nc.sync, nc.scalar, nc.gpsimd]

        def dma_xs(nm, src, sc, eng, split=False, eng2=None):
            t = p_xs.tile([128, DMT * SC], dmm, tag="xs", name=f"xs_{nm}{sc}")
            if split:
                h = DMT * SC // 2
                eng.dma_start(out=t[:, 0:h], in_=src[:, sc, 0:h])
                (eng2 or eng).dma_start(out=t[:, h:], in_=src[:, sc, h:])
            else:
                eng.dma_start(out=t[:], in_=src[:, sc, :])
            xs_state[(nm, sc)] = t

        # later weight DMAs (needed only once QK c0 chains are done)
        wv_sb = p_w.tile([128, DMT * GD], dmm, tag="wv", name="wv_sb")
        wo_sb = p_w.tile([128, NCH * D], dmm, tag="wo", name="wo_sb")

        qt_sb = [None] * GH
        kt_sb = [None] * GH
        ot_sb = [None] * NCH
        v_sb = [None] * KT

        for hg in range(GH):
            qt_sb[hg] = p_qt.tile([128, S], dmm, tag="qt", name=f"qt{hg}")
            kt_sb[hg] = p_kt.tile([128, S], dmm, tag="kt", name=f"kt{hg}")

        # ---- readiness counters (emission-order correctness) ----
        ready = {"v": 0}           # v_sb tiles emitted
        for c in range(NCH):
            ready[f"k{c}"] = 0     # sc chunks of K^T emitted for pair c
            ready[f"q{c}"] = 0

        fq = deque()

        def pump(budget):
            while fq and budget > 0:
                cost, fn = fq.popleft()
                fn()
                budget -= cost

        def require(key, n):
            while ready[key] < n:
                assert fq, f"filler queue empty but need {key}>={n}"
                cost, fn = fq.popleft()
                fn()

        # ---- projection chain token builders ----
        def qk_chain(nm, c, sc):
            """Tokens computing Q^T/K^T chunk (heads 2c,2c+1, s cols sc*512+)."""
            wsb = wq_sb if nm == "q" else wk_sb
            bsb = bq_sb if nm == "q" else bk_sb
            dsts = ([qt_sb[2 * c], qt_sb[2 * c + 1]] if nm == "q" else
                    [kt_sb[2 * c], kt_sb[2 * c + 1]])
            st = {}

            def mk(a0):
                def f():
                    if a0 == 0:
                        st["pp"] = p_pp.tile([128, SC], f32, tag="pp",
                                             name=f"pp{nm}{c}_{sc}")
                    xs = xs_state[(nm, sc)]
                    for a in (a0, a0 + 1):
                        nc.tensor.matmul(
                            out=st["pp"][:],
                            lhsT=wsb[:, a * GD + c * 128:a * GD + (c + 1) * 128],
                            rhs=xs[:, a * SC:(a + 1) * SC],
                            start=(a == 0), stop=(a == DMT - 1),
                        )
                return f

            def fin():
                ps = st["pp"]
                s0, s1 = sc * SC, (sc + 1) * SC
                # head 2c native rows 0-63; head 2c+1 native rows 64-127
                nc.vector.tensor_scalar_add(
                    out=dsts[0][0:DK, s0:s1], in0=ps[0:DK, :],
                    scalar1=bsb[0:DK, c:c + 1])
                nc.vector.tensor_scalar_add(
                    out=dsts[1][DK:128, s0:s1], in0=ps[DK:128, :],
                    scalar1=bsb[DK:128, c:c + 1])
                # duplicate into the other 64-row half (SBUF->SBUF DMA) so
                # scores can alternate PE row groups
                nc.sync.dma_start(out=dsts[0][DK:128, s0:s1],
                                  in_=dsts[0][0:DK, s0:s1])
                nc.sync.dma_start(out=dsts[1][0:DK, s0:s1],
                                  in_=dsts[1][DK:128, s0:s1])
                ready[f"{nm}{c}"] += 1

            return [(2, mk(0)), (2, mk(2)), (2, mk(4)), (2, mk(6)), (0, fin)]

        def v_chain(st_i):
            """Tokens computing V tile st_i: [128 s, GH, 65] (col 64 = ones)."""
            st = {}

            def mk(a0):
                def f():
                    if a0 == 0:
                        st["pp"] = p_pp.tile([128, GD], f32, tag="pp",
                                             name=f"ppv{st_i}")
                    xv_t = xs_state[("v", st_i // 4)]
                    sub = (st_i % 4) * 128
                    for a in (a0, a0 + 1):
                        nc.tensor.matmul(
                            out=st["pp"][:],
                            lhsT=xv_t[:, a * SC + sub:a * SC + sub + 128],
                            rhs=wv_sb[:, a * GD:(a + 1) * GD],
                            start=(a == 0), stop=(a == DMT - 1),
                        )
                return f

            def fin():
                vt = p_v.tile([128, GH, 65], dmm, tag="v", name=f"v{st_i}")
                nc.vector.tensor_copy(
                    out=vt[:, :, 0:DK],
                    in_=st["pp"][:].rearrange("p (h d) -> p h d", h=GH),
                )
                nc.vector.tensor_copy(
                    out=vt[:, :, DK:65],
                    in_=ones_sb.unsqueeze(1).broadcast_to([128, GH, 1]))
                v_sb[st_i] = vt
                ready["v"] += 1

            return [(2, mk(0)), (2, mk(2)), (2, mk(4)), (2, mk(6)), (0, fin)]

        def fin_chain(qt_i):
            """Output projection rows qt_i*128: out = sum_c ot[c]^T @ Wo[c]."""
            st = {}

            def mk(half, c0):
                def f():
                    if c0 == 0:
                        st[half] = p_pp.tile([128, SC], f32, tag="pp",
                                             name=f"ppf{qt_i}_{half}")
                    for c in (c0, c0 + 1):
                        nc.tensor.matmul(
                            out=st[half][:],
                            lhsT=ot_sb[c][:, qt_i * 128:(qt_i + 1) * 128],
                            rhs=wo_sb[:, c * D + half * 512:
                                      c * D + (half + 1) * 512],
                            start=(c == 0), stop=(c == NCH - 1),
                        )
                return f

            def cp(half):
                def f():
                    if "st" not in st:
                        st["st"] = p_st.tile([128, D], f32, tag="st",
                                             name=f"st{qt_i}")
                    nc.vector.tensor_copy(
                        out=st["st"][:, half * 512:(half + 1) * 512],
                        in_=st[half][:])
                    if half == 1:
                        eng = _dma_rr[qt_i % 3]
                        eng.dma_start(
                            out=out[qt_i * 128:(qt_i + 1) * 128, :],
                            in_=st["st"][:])
                return f

            return [(2, mk(0, 0)), (2, mk(0, 2)), (0, cp(0)),
                    (2, mk(1, 0)), (2, mk(1, 2)), (0, cp(1))]

        # ---- attention pass: heads 2c+hh, q cols [qc*QC, (qc+1)*QC) ----
        def attention_pass(c, qc, hh, ovts, fb=2, ovt_scalar=False,
                           pre=None):
            hg = 2 * c + hh
            require(f"q{c}", 2 * (qc + 1))
            pv_ps = p_pv.tile([65, QC], f32, tag="pv", name=f"pv{c}_{qc}_{hh}")
            pts = {}
            NP = KT // 2
            for ktp in range(NP + 1):
                if ktp == 5 and pre is not None:
                    # emit the NEXT pass's projection prerequisites here,
                    # mid-pass, instead of as a burst at the pass boundary
                    # (which would delay the next scores and starve EXP)
                    pre()
                if ktp < NP:
                    k0, k1 = 2 * ktp, 2 * ktp + 1
                    require(f"k{c}", (k1 * 128) // SC + 1)
                    pump(fb)
                    psA = p_ps.tile([128, QC], f32, tag="ps",
                                    name=f"psA{c}_{qc}_{hh}_{ktp}")
                    psB = p_ps.tile([128, QC], f32, tag="ps",
                                    name=f"psB{c}_{qc}_{hh}_{ktp}")
                    # kt even in PE rows 0:64, kt odd in rows 64:128: the two
                    # halves of one kt share a stationary (one LDWEIGHTS),
                    # and the odd-kt pair can stream in the other row group
                    for half in range(2):
                        q0 = qc * QC + half * 512
                        nc.tensor.matmul(
                            out=psA[:, half * 512:(half + 1) * 512],
                            lhsT=kt_sb[hg][0:DK, k0 * 128:(k0 + 1) * 128],
                            rhs=qt_sb[hg][0:DK, q0:q0 + 512],
                            start=True, stop=True, tile_position=(0, 0),
                        )
                    for half in range(2):
                        q0 = qc * QC + half * 512
                        nc.tensor.matmul(
                            out=psB[:, half * 512:(half + 1) * 512],
                            lhsT=kt_sb[hg][DK:128, k1 * 128:(k1 + 1) * 128],
                            rhs=qt_sb[hg][DK:128, q0:q0 + 512],
                            start=True, stop=True, tile_position=(DK, 0),
                        )
                    ptA = p_pt.tile([128, QC], dmm, tag="pt",
                                    name=f"ptA{c}_{qc}_{hh}_{ktp}")
                    ptB = p_pt.tile([128, QC], dmm, tag="pt",
                                    name=f"ptB{c}_{qc}_{hh}_{ktp}")
                    nc.scalar.activation(ptA[:], psA[:], Exp,
                                         bias=0.0, scale=0.125)
                    nc.scalar.activation(ptB[:], psB[:], Exp,
                                         bias=0.0, scale=0.125)
                    pts[k0], pts[k1] = ptA, ptB
                if ktp >= 1:
                    j0 = 2 * (ktp - 1)
                    require("v", j0 + 2)
                    pump(fb)
                    for j in (j0, j0 + 1):
                        for half in range(2):
                            nc.tensor.matmul(
                                out=pv_ps[:, half * 512:(half + 1) * 512],
                                lhsT=v_sb[j][:, hg, :],
                                rhs=pts[j][:, half * 512:(half + 1) * 512],
                                start=(j == 0), stop=(j == KT - 1),
                            )
            # evict PV psum right away to release its bank pair (NOT on
            # ScalarE: a copy there blocks the strict-FIFO ACT queue on the
            # last PV matmul, starving the next pass's EXPs)
            ovt = p_ov.tile([65, QC], f32, tag="ov", name=f"ov{c}_{qc}_{hh}")
            if ovt_scalar:
                nc.scalar.copy(out=ovt[:], in_=pv_ps[:])
            else:
                nc.vector.tensor_copy(out=ovt[:], in_=pv_ps[:])
            ovts[hh] = ovt

        def normalize_hh(c, qc, hh, ovt, last=False):
            # O^T = PV[0:64] * broadcast(1 / PV[64]); at the tail the sync
            # queue is busy with 512KB output DMAs, so route the tiny
            # reciprocal-scatter DMAs via the (idle) scalar queue there
            eng = nc.scalar if last else nc.sync
            zs = p_zr.tile([DK, QC // DK], f32, tag="zs",
                           name=f"zs{c}_{qc}_{hh}")
            eng.dma_start(out=zs[:], in_=ovt[DK:DK + 1, :])
            nc.vector.reciprocal(out=zs[:], in_=zs[:])
            zr = p_zr.tile([1, QC], f32, tag="zr", name=f"zr{c}_{qc}_{hh}")
            eng.dma_start(out=zr[:], in_=zs[:])

            rb = p_rb.tile([DK, QC], f32, tag="rb",
                           name=f"rb{c}_{qc}_{hh}")
            nc.gpsimd.partition_broadcast(rb[:], zr[:], channels=DK)
            if hh == 0:
                nc.vector.tensor_mul(
                    out=ot_sb[c][0:DK, qc * QC:(qc + 1) * QC],
                    in0=ovt[0:DK, :], in1=rb[:])
            else:
                tmp = p_rb.tile([DK, QC], dmm, tag="rb",
                                name=f"tmp{c}_{qc}")
                nc.vector.tensor_mul(out=tmp[:], in0=ovt[0:DK, :],
                                     in1=rb[:])
                nc.sync.dma_start(
                    out=ot_sb[c][DK:128, qc * QC:(qc + 1) * QC],
                    in_=tmp[:])

        def normalize(c, qc, ovts):
            for hh in range(2):
                normalize_hh(c, qc, hh, ovts[hh])

        # ---- PRE: minimal work to start attention ----
        dma_xs("k", xk, 0, nc.sync, split=True, eng2=nc.scalar)
        dma_xs("q", xq, 0, nc.sync, split=True, eng2=nc.gpsimd)
        dma_xs("q", xq, 1, nc.sync, split=True, eng2=nc.scalar)
        for tok in qk_chain("k", 0, 0):
            tok[1]()
        for tok in qk_chain("q", 0, 0):
            tok[1]()
        nc.sync.dma_start(out=wv_sb[:, 0:HW], in_=wv[:, 0:HW])
        nc.scalar.dma_start(out=wv_sb[:, HW:], in_=wv[:, HW:])
        dma_xs("v", xv, 0, nc.sync, split=True, eng2=nc.gpsimd)
        for tok in qk_chain("q", 0, 1):
            tok[1]()

        # ---- filler queue: everything else, in consumption order ----
        def push_xs(nm, src, sc, eng):
            fq.append((0, lambda: dma_xs(nm, src, sc, eng)))

        # pair 0 (qc0 passes): V tiles + remaining K c0 chunks + Q c0 qc1
        fq.extend(v_chain(0))
        fq.extend(v_chain(1))
        push_xs("k", xk, 1, nc.sync)
        fq.extend(v_chain(2))
        fq.extend(qk_chain("k", 0, 1))
        fq.extend(v_chain(3))
        push_xs("v", xv, 1, nc.gpsimd)
        push_xs("k", xk, 2, nc.sync)
        fq.extend(v_chain(4))
        fq.extend(qk_chain("k", 0, 2))
        fq.extend(v_chain(5))
        push_xs("k", xk, 3, nc.sync)
        fq.extend(v_chain(6))
        fq.extend(qk_chain("k", 0, 3))
        fq.extend(v_chain(7))
        push_xs("v", xv, 2, nc.gpsimd)
        for st_i in range(8, 12):
            fq.extend(v_chain(st_i))
        push_xs("v", xv, 3, nc.gpsimd)
        for st_i in range(12, 16):
            if st_i == 12:
                push_xs("q", xq, 2, nc.scalar)
            fq.extend(v_chain(st_i))
        fq.append((0, lambda: nc.scalar.dma_start(out=wo_sb[:], in_=wo)))
        fq.extend(qk_chain("q", 0, 2))
        push_xs("q", xq, 3, nc.scalar)
        fq.extend(qk_chain("q", 0, 3))

        # pair 0 qc1 passes: K/Q chains for c=1
        for sc in range(NSC):
            push_xs("k", xk, sc, nc.sync)
            fq.extend(qk_chain("k", 1, sc))
        for sc in range(2):
            push_xs("q", xq, sc, nc.scalar)
            fq.extend(qk_chain("q", 1, sc))
        # pair 1: chains for c=2 (+ q c1 qc1)
        for sc in range(2, NSC):
            push_xs("q", xq, sc, nc.scalar)
            fq.extend(qk_chain("q", 1, sc))
        for sc in range(NSC):
            push_xs("k", xk, sc, nc.sync)
            fq.extend(qk_chain("k", 2, sc))
        for sc in range(2):
            push_xs("q", xq, sc, nc.scalar)
            fq.extend(qk_chain("q", 2, sc))
        # pair 2: chains for c=3 (+ q c2 qc1)
        for sc in range(2, NSC):
            push_xs("q", xq, sc, nc.scalar)
            fq.extend(qk_chain("q", 2, sc))
        for sc in range(NSC):
            push_xs("k", xk, sc, nc.sync)
            fq.extend(qk_chain("k", 3, sc))
        for sc in range(NSC):
            push_xs("q", xq, sc, nc.scalar)
            fq.extend(qk_chain("q", 3, sc))

        # ---- attention pairs ----
        for c in range(NCH):
            ot_sb[c] = p_ot.tile([128, S], dmm, tag="ot", name=f"ot{c}")
            for qc in range(NQC):
                ovts = [None, None]
                if c == NCH - 1 and qc == NQC - 1:
                    # last chunk: hh=1 first so its long normalize chain
                    # (broadcast + mul + SBUF copy) overlaps the hh=0 pass,
                    # leaving only hh=0's short chain for the tail
                    attention_pass(c, qc, 1, ovts, fb=3)
                    normalize_hh(c, qc, 1, ovts[1], last=True)
                    attention_pass(c, qc, 0, ovts, fb=3, ovt_scalar=True)
                    break
                fb = 1 if c == 0 else (2 if c == 1 else 3)
                # next pass's start-requirements, prefetched mid-pass
                if qc + 1 < NQC:
                    cn, qn = c, qc + 1
                else:
                    cn, qn = c + 1, 0

                def mkpre(cn=cn, qn=qn):
                    require(f"q{cn}", 2 * (qn + 1))
                    require(f"k{cn}", 1)
                attention_pass(c, qc, 0, ovts, fb=fb)
                attention_pass(c, qc, 1, ovts, fb=fb, pre=mkpre)
                normalize(c, qc, ovts)
                if c == NCH - 1 and qc == 0:
                    # final projections for q rows 0:1024 become the fillers
                    # of pair 3's qc1 passes
                    for qt_i in range(8):
                        fq.extend(fin_chain(qt_i))
        pump(1 << 30)

        # ---- tail: pipelined final projections for q rows 1024:2048,
        # interleaved with the last pass's normalization so the PE never
        # waits on the (long) reciprocal/broadcast chain ----
        class TailFin:
            def __init__(self, qt_i, use_pp):
                self.qt_i = qt_i
                self.use_pp = use_pp
                self.pool = p_ps
                self.halves = None
                self.whole = None

            def _alloc(self):
                qt_i = self.qt_i
                if self.use_pp:
                    t0 = p_pp.tile([128, SC], f32, tag="pp",
                                   name=f"fpp{qt_i}_0")
                    t1 = p_pp.tile([128, SC], f32, tag="pp",
                                   name=f"fpp{qt_i}_1")
                    self.halves = [t0[:], t1[:]]
                else:
                    tag = "ps" if self.pool is p_ps else "pv"
                    t = self.pool.tile([128, D], f32, tag=tag,
                                       name=f"fps{qt_i}")
                    self.halves = [t[:, 0:512], t[:, 512:1024]]
                    self.whole = t

            def mm(self, c_list):
                if self.halves is None:
                    self._alloc()
                for half in range(2):
                    for cc in c_list:
                        nc.tensor.matmul(
                            out=self.halves[half],
                            lhsT=ot_sb[cc][:, self.qt_i * 128:
                                           (self.qt_i + 1) * 128],
                            rhs=wo_sb[:, cc * D + half * 512:
                                      cc * D + (half + 1) * 512],
                            start=(cc == 0), stop=(cc == NCH - 1),
                        )

            def close(self):
                self.mm([3])
                stt = p_st.tile([128, D], f32, tag="st",
                                name=f"stt{self.qt_i}")
                if self.whole is not None:
                    nc.vector.tensor_copy(out=stt[:], in_=self.whole[:])
                else:
                    for half in range(2):
                        nc.vector.tensor_copy(
                            out=stt[:, half * 512:(half + 1) * 512],
                            in_=self.halves[half])
                _dma_rr[self.qt_i % 3].dma_start(
                    out=out[self.qt_i * 128:(self.qt_i + 1) * 128, :],
                    in_=stt[:])

        fins = {q: TailFin(q, use_pp=(q in (10, 13))) for q in range(8, KT)}
        fins[11].pool = p_pv
        fins[8].mm([0, 1, 2])
        normalize_hh(NCH - 1, NQC - 1, 0, ovts[0], last=True)
        fins[9].mm([0, 1, 2])
        fins[10].mm([0, 1, 2])
        fins[11].mm([0, 1, 2])
        for a, b in ((None, 8), (12, 9), (13, 10), (14, 11), (15, 12),
                     (None, 13), (None, 14), (None, 15)):
            if a is not None:
                fins[a].mm([0, 1, 2])
            fins[b].close()

    nc.compile()
    return nc


def get_program():
    if "nc" not in _CACHE:
        _CACHE["nc"] = _build_program()
    return _CACHE["nc"]


def _relayout_x(xt):
    # x^T [D, S] -> [128, NSC, DMT*SC]: [p, sc, a*SC+s] = xT[a*128+p, sc*SC+s]
    return np.ascontiguousarray(
        xt.reshape(DMT, 128, NSC, SC).transpose(1, 2, 0, 3)
    ).reshape(128, NSC, DMT * SC)


def make_in_maps(inputs):
    dt = _np_mm_dtype()
    q = np.asarray(inputs["query"], np.float32)
    k = np.asarray(inputs["key"], np.float32)
    v = np.asarray(inputs["value"], np.float32)
    Wq = np.asarray(inputs["Wq"], np.float32)
    Wk = np.asarray(inputs["Wk"], np.float32)
    Wv = np.asarray(inputs["Wv"], np.float32)
    Wo = np.asarray(inputs["Wo"], np.float32)
    bq = np.asarray(inputs["bq"], np.float32)
    bk = np.asarray(inputs["bk"], np.float32)
    in_maps = []
    xs_cache = {}
    for b in range(B):
        xs_cache[b] = {
            "xq": _relayout_x(q[b].T.astype(dt)),
            "xk": _relayout_x(k[b].T.astype(dt)),
            "xv": _relayout_x(v[b].T.astype(dt)),
        }
    for core in range(NCORES):
        b, g = core // 2, core % 2
        sl = slice(g * GD, (g + 1) * GD)
        # weights [D, GD] -> [128, DMT*GD]
        wq_a = np.ascontiguousarray(
            Wq[:, sl].astype(dt).reshape(DMT, 128, GD).transpose(1, 0, 2)
        ).reshape(128, DMT * GD)
        wk_a = np.ascontiguousarray(
            Wk[:, sl].astype(dt).reshape(DMT, 128, GD).transpose(1, 0, 2)
        ).reshape(128, DMT * GD)
        wv_a = np.ascontiguousarray(
            Wv[:, sl].astype(dt).reshape(DMT, 128, GD).transpose(1, 0, 2)
        ).reshape(128, DMT * GD)
        # wo [GD, D] -> [128, NCH*D]
        wo_a = np.ascontiguousarray(
            Wo[sl, :].astype(dt).reshape(NCH, 128, D).transpose(1, 0, 2)
        ).reshape(128, NCH * D)
        in_maps.append({
            "xq": xs_cache[b]["xq"],
            "xk": xs_cache[b]["xk"],
            "xv": xs_cache[b]["xv"],
            "wq": wq_a, "wk": wk_a, "wv": wv_a, "wo": wo_a,
            "bq": np.ascontiguousarray(bq[sl].reshape(NCH, 128).T),
            "bk": np.ascontiguousarray(bk[sl].reshape(NCH, 128).T),
        })
    return in_maps


def combine_outputs(results, inputs):
    Wo = np.asarray(inputs["Wo"], np.float32)
    bv = np.asarray(inputs["bv"], np.float32)
    bo = np.asarray(inputs["bo"], np.float32)
    out = np.empty((B, S, D), np.float32)
    for b in range(B):
        out[b] = results[2 * b]["out"] + results[2 * b + 1]["out"]
    out += bv @ Wo + bo
    return out


def kernel(**inputs):
    from concourse.bass_utils import run_bass_kernel_spmd
    nc = get_program()
    in_maps = make_in_maps(inputs)
    res = run_bass_kernel_spmd(nc, in_maps, list(range(NCORES)))
    return combine_outputs(res.results, inputs)


# revision 36
# speedup vs baseline: 1.0329x; 1.0272x over previous
"""Multi-head attention (B=4, S=2048, D=1024, H=16) on 8 trn2 NeuronCores.

Sharding: core = (batch b, head-group g) with b = core//2, g = core%2.
Each core handles one batch and 8 heads (512 of the 1024 d_model dims).

v2 structure (per core):
  - host pre-relayouts x^T / weights so every device DMA is 128 descriptors
    of contiguous >=4KB runs (kills HWDGE DIRECT2D descriptor-gen cost)
  - QK^T scores are computed in kt PAIRS: kt even uses PE rows 0:64, kt odd
    rows 64:128 (the per-head q/k tiles hold the 64 dims twice), emitted
    interleaved so the two 64-row matmuls stream CONCURRENTLY in separate
    PE row groups
  - softmax exp (ACT) is pipelined one kt-pair deep: iteration i emits
    scores for pair i and PV matmuls for pair i-1, so EXP latency is never
    on the PE critical path
  - PSUM: scores pool 2x[128,1024] (4 banks) + PV accum [65,1024] (2) +
    projection pool 2x[128,512] (2) = all 8 banks; projections never steal
    scores buffers
  - projections (QKV, V, output) are emitted through a fine-grained filler
    queue (~2 matmuls per token) pumped between attention matmuls, with
    readiness counters forcing emission before first use
  - softmax denominator comes out of the PV matmul via a ones-column
    appended to V; normalization (reciprocal + broadcast + mul) runs on
    DVE/gpsimd off the critical path
  - output projection vs Wo[g*512:(g+1)*512, :] gives a partial [2048,1024]
    per core; host sums the two group partials per batch, adds bv@Wo + bo
"""

import os
import numpy as np
from collections import deque
from contextlib import ExitStack

B = 4
S = 2048
D = 1024
H = 16
DK = 64
NCORES = 8
GH = 8           # heads per core (group)
GD = GH * DK     # 512 head dims per core
NCH = GD // 128  # 4 chunks of 128 output dims
KT = S // 128    # 16 key tiles
QC = 1024        # q chunk width for attention
NQC = S // QC    # 2
SC = 512         # s chunk width for projections
NSC = S // SC    # 4
DMT = D // 128   # 8 d_model tiles

MM_DT = os.environ.get("MM_DT", "bf16")  # "bf16" | "f32r"

_CACHE = {}


def _np_mm_dtype():
    if MM_DT == "bf16":
        import ml_dtypes
        return ml_dtypes.bfloat16
    return np.float32


def _build_program():
    import concourse.mybir as mybir
    import concourse.tile as tile
    from concourse import bacc

    f32 = mybir.dt.float32
    dmm = mybir.dt.bfloat16 if MM_DT == "bf16" else mybir.dt.float32r

    nc = bacc.Bacc("TRN2", target_bir_lowering=False, debug=False,
                   num_devices=NCORES)

    # host-relayout inputs: x{q,k,v} as [128, NSC, DMT*SC] where
    # [p, sc, a*SC+s] = x^T[a*128+p, sc*SC+s]  (contiguous 8KB per (p,sc))
    xq = nc.dram_tensor("xq", [128, NSC, DMT * SC], dmm,
                        kind="ExternalInput").ap()
    xk = nc.dram_tensor("xk", [128, NSC, DMT * SC], dmm,
                        kind="ExternalInput").ap()
    xv = nc.dram_tensor("xv", [128, NSC, DMT * SC], dmm,
                        kind="ExternalInput").ap()
    # weights as [128, DMT*GD]: [p, a*GD+d] = W[a*128+p, d]
    wq = nc.dram_tensor("wq", [128, DMT * GD], dmm, kind="ExternalInput").ap()
    wk = nc.dram_tensor("wk", [128, DMT * GD], dmm, kind="ExternalInput").ap()
    wv = nc.dram_tensor("wv", [128, DMT * GD], dmm, kind="ExternalInput").ap()
    # wo as [128, NCH*D]: [p, c*D+n] = Wo[c*128+p, n]
    wo = nc.dram_tensor("wo", [128, NCH * D], dmm, kind="ExternalInput").ap()
    # biases as [128, NCH]: [p, a] = b[a*128+p]
    bq = nc.dram_tensor("bq", [128, NCH], f32, kind="ExternalInput").ap()
    bk = nc.dram_tensor("bk", [128, NCH], f32, kind="ExternalInput").ap()
    out = nc.dram_tensor("out", [S, D], f32, kind="ExternalOutput").ap()

    Exp = mybir.ActivationFunctionType.Exp

    with tile.TileContext(nc) as tc, ExitStack() as ctx:
        # ---- SBUF pools ----
        p_qt = ctx.enter_context(tc.tile_pool(name="qt", bufs=GH))
        p_kt = ctx.enter_context(tc.tile_pool(name="kt", bufs=GH))
        p_v = ctx.enter_context(tc.tile_pool(name="v", bufs=KT))
        p_ot = ctx.enter_context(tc.tile_pool(name="ot", bufs=NCH))
        p_w = ctx.enter_context(tc.tile_pool(name="w", bufs=1))
        p_bias = ctx.enter_context(tc.tile_pool(name="bias", bufs=1))
        p_xs = ctx.enter_context(tc.tile_pool(name="xs", bufs=3))
        p_pt = ctx.enter_context(tc.tile_pool(name="pt", bufs=6))
        p_zr = ctx.enter_context(tc.tile_pool(name="zr", bufs=2))
        p_rb = ctx.enter_context(tc.tile_pool(name="rb", bufs=2))
        p_st = ctx.enter_context(tc.tile_pool(name="st", bufs=2))
        p_ov = ctx.enter_context(tc.tile_pool(name="ov", bufs=3))
        # ---- PSUM: 2*2 + 1*2 + 2*1 = 8 banks ----
        p_ps = ctx.enter_context(tc.tile_pool(name="ps", bufs=2, space="PSUM"))
        p_pv = ctx.enter_context(tc.tile_pool(name="pv", bufs=1, space="PSUM"))
        p_pp = ctx.enter_context(tc.tile_pool(name="pp", bufs=2, space="PSUM"))

        # ---- warm the DMA rings with tiny transfers so the first real
        # DMA doesn't pay ring-init latency ----
        wrmd = p_bias.tile([128, 4], f32, tag="wrmd")
        nc.sync.dma_start(out=wrmd[:, 0:1], in_=bq[:, 0:1])
        nc.scalar.dma_start(out=wrmd[:, 1:2], in_=bq[:, 0:1])
        nc.gpsimd.dma_start(out=wrmd[:, 2:3], in_=bq[:, 0:1])

        # ---- initial DMAs, spread across the 3 DMA-issuing queues ----
        HW = DMT * GD // 2
        wk_sb = p_w.tile([128, DMT * GD], dmm, tag="wk", name="wk_sb")
        nc.scalar.dma_start(out=wk_sb[:, 0:HW], in_=wk[:, 0:HW])
        nc.gpsimd.dma_start(out=wk_sb[:, HW:], in_=wk[:, HW:])
        wq_sb = p_w.tile([128, DMT * GD], dmm, tag="wq", name="wq_sb")
        nc.scalar.dma_start(out=wq_sb[:, 0:HW], in_=wq[:, 0:HW])
        nc.gpsimd.dma_start(out=wq_sb[:, HW:], in_=wq[:, HW:])
        bq_sb = p_bias.tile([128, NCH], f32, tag="bq")
        nc.gpsimd.dma_start(out=bq_sb[:], in_=bq)
        bk_sb = p_bias.tile([128, NCH], f32, tag="bk")
        nc.gpsimd.dma_start(out=bk_sb[:], in_=bk)
        ones_sb = p_bias.tile([128, 1], f32, tag="ones")
        nc.vector.memset(ones_sb[:], 1.0)
        warm = p_bias.tile([128, 512], dmm, tag="warm")
        nc.vector.memset(warm[:], 0.0)
        wps = p_pp.tile([128, SC], f32, tag="pp", name="wps")
        for i in range(4):
            nc.tensor.matmul(out=wps[:], lhsT=warm[:, 0:128], rhs=warm[:],
                             start=(i == 0), stop=(i == 3))

        xs_state = {}
        _dma_rr = [nc.sync, nc.scalar, nc.gpsimd]

        def dma_xs(nm, src, sc, eng, split=False, eng2=None):
            t = p_xs.tile([128, DMT * SC], dmm, tag="xs", name=f"xs_{nm}{sc}")
            if split:
                h = DMT * SC // 2
                eng.dma_start(out=t[:, 0:h], in_=src[:, sc, 0:h])
                (eng2 or eng).dma_start(out=t[:, h:], in_=src[:, sc, h:])
            else:
                eng.dma_start(out=t[:], in_=src[:, sc, :])
            xs_state[(nm, sc)] = t

        # later weight DMAs (needed only once QK c0 chains are done)
        wv_sb = p_w.tile([128, DMT * GD], dmm, tag="wv", name="wv_sb")
        wo_sb = p_w.tile([128, NCH * D], dmm, tag="wo", name="wo_sb")

        qt_sb = [None] * GH
        kt_sb = [None] * GH
        ot_sb = [None] * NCH
        v_sb = [None] * KT

        for hg in range(GH):
            qt_sb[hg] = p_qt.tile([128, S], dmm, tag="qt", name=f"qt{hg}")
            kt_sb[hg] = p_kt.tile([128, S], dmm, tag="kt", name=f"kt{hg}")

        # ---- readiness counters (emission-order correctness) ----
        ready = {"v": 0}           # v_sb tiles emitted
        for c in range(NCH):
            ready[f"k{c}"] = 0     # sc chunks of K^T emitted for pair c
            ready[f"q{c}"] = 0

        fq = deque()

        def pump(budget):
            while fq and budget > 0:
                cost, fn = fq.popleft()
                fn()
                budget -= cost

        def require(key, n):
            while ready[key] < n:
                assert fq, f"filler queue empty but need {key}>={n}"
                cost, fn = fq.popleft()
                fn()

        # ---- projection chain token builders ----
        def qk_chain(nm, c, sc):
            """Tokens computing Q^T/K^T chunk (heads 2c,2c+1, s cols sc*512+)."""
            wsb = wq_sb if nm == "q" else wk_sb
            bsb = bq_sb if nm == "q" else bk_sb
            dsts = ([qt_sb[2 * c], qt_sb[2 * c + 1]] if nm == "q" else
                    [kt_sb[2 * c], kt_sb[2 * c + 1]])
            st = {}

            def mk(a0):
                def f():
                    if a0 == 0:
                        st["pp"] = p_pp.tile([128, SC], f32, tag="pp",
                                             name=f"pp{nm}{c}_{sc}")
                    xs = xs_state[(nm, sc)]
                    for a in (a0, a0 + 1):
                        nc.tensor.matmul(
                            out=st["pp"][:],
                            lhsT=wsb[:, a * GD + c * 128:a * GD + (c + 1) * 128],
                            rhs=xs[:, a * SC:(a + 1) * SC],
                            start=(a == 0), stop=(a == DMT - 1),
                        )
                return f

            def fin():
                ps = st["pp"]
                s0, s1 = sc * SC, (sc + 1) * SC
                # head 2c native rows 0-63; head 2c+1 native rows 64-127
                nc.vector.tensor_scalar_add(
                    out=dsts[0][0:DK, s0:s1], in0=ps[0:DK, :],
                    scalar1=bsb[0:DK, c:c + 1])
                nc.vector.tensor_scalar_add(
                    out=dsts[1][DK:128, s0:s1], in0=ps[DK:128, :],
                    scalar1=bsb[DK:128, c:c + 1])
                # duplicate into the other 64-row half (SBUF->SBUF DMA) so
                # scores can alternate PE row groups
                nc.sync.dma_start(out=dsts[0][DK:128, s0:s1],
                                  in_=dsts[0][0:DK, s0:s1])
                nc.sync.dma_start(out=dsts[1][0:DK, s0:s1],
                                  in_=dsts[1][DK:128, s0:s1])
                ready[f"{nm}{c}"] += 1

            return [(2, mk(0)), (2, mk(2)), (2, mk(4)), (2, mk(6)), (0, fin)]

        def v_chain(st_i):
            """Tokens computing V tile st_i: [128 s, GH, 65] (col 64 = ones)."""
            st = {}

            def mk(a0):
                def f():
                    if a0 == 0:
                        st["pp"] = p_pp.tile([128, GD], f32, tag="pp",
                                             name=f"ppv{st_i}")
                    xv_t = xs_state[("v", st_i // 4)]
                    sub = (st_i % 4) * 128
                    for a in (a0, a0 + 1):
                        nc.tensor.matmul(
                            out=st["pp"][:],
                            lhsT=xv_t[:, a * SC + sub:a * SC + sub + 128],
                            rhs=wv_sb[:, a * GD:(a + 1) * GD],
                            start=(a == 0), stop=(a == DMT - 1),
                        )
                return f

            def fin():
                vt = p_v.tile([128, GH, 65], dmm, tag="v", name=f"v{st_i}")
                nc.vector.tensor_copy(
                    out=vt[:, :, 0:DK],
                    in_=st["pp"][:].rearrange("p (h d) -> p h d", h=GH),
                )
                nc.vector.tensor_copy(
                    out=vt[:, :, DK:65],
                    in_=ones_sb.unsqueeze(1).broadcast_to([128, GH, 1]))
                v_sb[st_i] = vt
                ready["v"] += 1

            return [(2, mk(0)), (2, mk(2)), (2, mk(4)), (2, mk(6)), (0, fin)]

        def fin_chain(qt_i):
            """Output projection rows qt_i*128: out = sum_c ot[c]^T @ Wo[c]."""
            st = {}

            def mk(half, c0):
                def f():
                    if c0 == 0:
                        st[half] = p_pp.tile([128, SC], f32, tag="pp",
                                             name=f"ppf{qt_i}_{half}")
                    for c in (c0, c0 + 1):
                        nc.tensor.matmul(
                            out=st[half][:],
                            lhsT=ot_sb[c][:, qt_i * 128:(qt_i + 1) * 128],
                            rhs=wo_sb[:, c * D + half * 512:
                                      c * D + (half + 1) * 512],
                            start=(c == 0), stop=(c == NCH - 1),
                        )
                return f

            def cp(half):
                def f():
                    if "st" not in st:
                        st["st"] = p_st.tile([128, D], f32, tag="st",
                                             name=f"st{qt_i}")
                    nc.vector.tensor_copy(
                        out=st["st"][:, half * 512:(half + 1) * 512],
                        in_=st[half][:])
                    if half == 1:
                        eng = _dma_rr[qt_i % 3]
                        eng.dma_start(
                            out=out[qt_i * 128:(qt_i + 1) * 128, :],
                            in_=st["st"][:])
                return f

            return [(2, mk(0, 0)), (2, mk(0, 2)), (0, cp(0)),
                    (2, mk(1, 0)), (2, mk(1, 2)), (0, cp(1))]

        # ---- attention pass: heads 2c+hh, q cols [qc*QC, (qc+1)*QC) ----
        def attention_pass(c, qc, hh, ovts, fb=2, ovt_scalar=False,
                           pre=None):
            hg = 2 * c + hh
            require(f"q{c}", 2 * (qc + 1))
            pv_ps = p_pv.tile([65, QC], f32, tag="pv", name=f"pv{c}_{qc}_{hh}")
            pts = {}
            NP = KT // 2
            for ktp in range(NP + 1):
                if ktp == 5 and pre is not None:
                    # emit the NEXT pass's projection prerequisites here,
                    # mid-pass, instead of as a burst at the pass boundary
                    # (which would delay the next scores and starve EXP)
                    pre()
                if ktp < NP:
                    k0, k1 = 2 * ktp, 2 * ktp + 1
                    require(f"k{c}", (k1 * 128) // SC + 1)
                    pump(fb)
                    psA = p_ps.tile([128, QC], f32, tag="ps",
                                    name=f"psA{c}_{qc}_{hh}_{ktp}")
                    psB = p_ps.tile([128, QC], f32, tag="ps",
                                    name=f"psB{c}_{qc}_{hh}_{ktp}")
                    # kt even in PE rows 0:64, kt odd in rows 64:128: the two
                    # halves of one kt share a stationary (one LDWEIGHTS),
                    # and the odd-kt pair can stream in the other row group
                    for half in range(2):
                        q0 = qc * QC + half * 512
                        nc.tensor.matmul(
                            out=psA[:, half * 512:(half + 1) * 512],
                            lhsT=kt_sb[hg][0:DK, k0 * 128:(k0 + 1) * 128],
                            rhs=qt_sb[hg][0:DK, q0:q0 + 512],
                            start=True, stop=True, tile_position=(0, 0),
                        )
                    for half in range(2):
                        q0 = qc * QC + half * 512
                        nc.tensor.matmul(
                            out=psB[:, half * 512:(half + 1) * 512],
                            lhsT=kt_sb[hg][DK:128, k1 * 128:(k1 + 1) * 128],
                            rhs=qt_sb[hg][DK:128, q0:q0 + 512],
                            start=True, stop=True, tile_position=(DK, 0),
                        )
                    ptA = p_pt.tile([128, QC], dmm, tag="pt",
                                    name=f"ptA{c}_{qc}_{hh}_{ktp}")
                    ptB = p_pt.tile([128, QC], dmm, tag="pt",
                                    name=f"ptB{c}_{qc}_{hh}_{ktp}")
                    nc.scalar.activation(ptA[:], psA[:], Exp,
                                         bias=0.0, scale=0.125)
                    nc.scalar.activation(ptB[:], psB[:], Exp,
                                         bias=0.0, scale=0.125)
                    pts[k0], pts[k1] = ptA, ptB
                if ktp >= 1:
                    j0 = 2 * (ktp - 1)
                    require("v", j0 + 2)
                    pump(fb)
                    for j in (j0, j0 + 1):
                        for half in range(2):
                            nc.tensor.matmul(
                                out=pv_ps[:, half * 512:(half + 1) * 512],
                                lhsT=v_sb[j][:, hg, :],
                                rhs=pts[j][:, half * 512:(half + 1) * 512],
                                start=(j == 0), stop=(j == KT - 1),
                            )
            # evict PV psum right away to release its bank pair (NOT on
            # ScalarE: a copy there blocks the strict-FIFO ACT queue on the
            # last PV matmul, starving the next pass's EXPs)
            ovt = p_ov.tile([65, QC], f32, tag="ov", name=f"ov{c}_{qc}_{hh}")
            if ovt_scalar:
                nc.scalar.copy(out=ovt[:], in_=pv_ps[:])
            else:
                nc.vector.tensor_copy(out=ovt[:], in_=pv_ps[:])
            ovts[hh] = ovt

        def normalize_hh(c, qc, hh, ovt, last=False):
            # O^T = PV[0:64] * broadcast(1 / PV[64]); at the tail the sync
            # queue is busy with 512KB output DMAs, so route the tiny
            # reciprocal-scatter DMAs via the (idle) scalar queue there
            eng = nc.scalar if last else nc.sync
            zs = p_zr.tile([DK, QC // DK], f32, tag="zs",
                           name=f"zs{c}_{qc}_{hh}")
            eng.dma_start(out=zs[:], in_=ovt[DK:DK + 1, :])
            nc.vector.reciprocal(out=zs[:], in_=zs[:])
            zr = p_zr.tile([1, QC], f32, tag="zr", name=f"zr{c}_{qc}_{hh}")
            eng.dma_start(out=zr[:], in_=zs[:])

            rb = p_rb.tile([DK, QC], f32, tag="rb",
                           name=f"rb{c}_{qc}_{hh}")
            nc.gpsimd.partition_broadcast(rb[:], zr[:], channels=DK)
            if hh == 0:
                nc.vector.tensor_mul(
                    out=ot_sb[c][0:DK, qc * QC:(qc + 1) * QC],
                    in0=ovt[0:DK, :], in1=rb[:])
            else:
                tmp = p_rb.tile([DK, QC], dmm, tag="rb",
                                name=f"tmp{c}_{qc}")
                nc.vector.tensor_mul(out=tmp[:], in0=ovt[0:DK, :],
                                     in1=rb[:])
                nc.sync.dma_start(
                    out=ot_sb[c][DK:128, qc * QC:(qc + 1) * QC],
                    in_=tmp[:])

        def normalize(c, qc, ovts):
            for hh in range(2):
                normalize_hh(c, qc, hh, ovts[hh])

        # ---- PRE: minimal work to start attention ----
        dma_xs("k", xk, 0, nc.sync, split=True, eng2=nc.scalar)
        dma_xs("q", xq, 0, nc.sync, split=True, eng2=nc.gpsimd)
        dma_xs("q", xq, 1, nc.sync, split=True, eng2=nc.scalar)
        for tok in qk_chain("k", 0, 0):
            tok[1]()
        for tok in qk_chain("q", 0, 0):
            tok[1]()
        nc.sync.dma_start(out=wv_sb[:, 0:HW], in_=wv[:, 0:HW])
        nc.scalar.dma_start(out=wv_sb[:, HW:], in_=wv[:, HW:])
        dma_xs("v", xv, 0, nc.sync, split=True, eng2=nc.gpsimd)
        for tok in qk_chain("q", 0, 1):
            tok[1]()

        # ---- filler queue: everything else, in consumption order ----
        def push_xs(nm, src, sc, eng):
            fq.append((0, lambda: dma_xs(nm, src, sc, eng)))

        # pair 0 (qc0 passes): V tiles + remaining K c0 chunks + Q c0 qc1
        fq.extend(v_chain(0))
        fq.extend(v_chain(1))
        push_xs("k", xk, 1, nc.sync)
        fq.extend(v_chain(2))
        fq.extend(qk_chain("k", 0, 1))
        fq.extend(v_chain(3))
        push_xs("v", xv, 1, nc.gpsimd)
        push_xs("k", xk, 2, nc.sync)
        fq.extend(v_chain(4))
        fq.extend(qk_chain("k", 0, 2))
        fq.extend(v_chain(5))
        push_xs("k", xk, 3, nc.sync)
        fq.extend(v_chain(6))
        fq.extend(qk_chain("k", 0, 3))
        fq.extend(v_chain(7))
        push_xs("v", xv, 2, nc.gpsimd)
        for st_i in range(8, 12):
            fq.extend(v_chain(st_i))
        push_xs("v", xv, 3, nc.gpsimd)
        for st_i in range(12, 16):
            if st_i == 12:
                push_xs("q", xq, 2, nc.scalar)
            fq.extend(v_chain(st_i))
        fq.append((0, lambda: nc.scalar.dma_start(out=wo_sb[:], in_=wo)))
        fq.extend(qk_chain("q", 0, 2))
        push_xs("q", xq, 3, nc.scalar)
        fq.extend(qk_chain("q", 0, 3))

        # pair 0 qc1 passes: K/Q chains for c=1
        for sc in range(NSC):
            push_xs("k", xk, sc, nc.sync)
            fq.extend(qk_chain("k", 1, sc))
        for sc in range(2):
            push_xs("q", xq, sc, nc.scalar)
            fq.extend(qk_chain("q", 1, sc))
        # pair 1: chains for c=2 (+ q c1 qc1)
        for sc in range(2, NSC):
            push_xs("q", xq, sc, nc.scalar)
            fq.extend(qk_chain("q", 1, sc))
        for sc in range(NSC):
            push_xs("k", xk, sc, nc.sync)
            fq.extend(qk_chain("k", 2, sc))
        for sc in range(2):
            push_xs("q", xq, sc, nc.scalar)
            fq.extend(qk_chain("q", 2, sc))
        # pair 2: chains for c=3 (+ q c2 qc1)
        for sc in range(2, NSC):
            push_xs("q", xq, sc, nc.scalar)
            fq.extend(qk_chain("q", 2, sc))
        for sc in range(NSC):
            push_xs("k", xk, sc, nc.sync)
            fq.extend(qk_chain("k", 3, sc))
        for sc in range(NSC):
            push_xs("q", xq, sc, nc.scalar)
            fq.extend(qk_chain("q", 3, sc))

        # ---- attention pairs ----
        for c in range(NCH):
            ot_sb[c] = p_ot.tile([128, S], dmm, tag="ot", name=f"ot{c}")
            for qc in range(NQC):
                ovts = [None, None]
                if c == NCH - 1 and qc == NQC - 1:
                    # last chunk: hh=1 first so its long normalize chain
                    # (broadcast + mul + SBUF copy) overlaps the hh=0 pass,
                    # leaving only hh=0's short chain for the tail
                    attention_pass(c, qc, 1, ovts, fb=3)
                    normalize_hh(c, qc, 1, ovts[1], last=True)
                    attention_pass(c, qc, 0, ovts, fb=3, ovt_scalar=True)
                    break
                fb = 1 if c == 0 else (2 if c == 1 else 3)
                # next pass's start-requirements, prefetched mid-pass
                if qc + 1 < NQC:
                    cn, qn = c, qc + 1
                else:
                    cn, qn = c + 1, 0

                def mkpre(cn=cn, qn=qn):
                    require(f"q{cn}", 2 * (qn + 1))
                    require(f"k{cn}", 1)
                attention_pass(c, qc, 0, ovts, fb=fb)
                attention_pass(c, qc, 1, ovts, fb=fb, pre=mkpre)
                normalize(c, qc, ovts)
                if c == NCH - 1 and qc == 0:
                    # final projections for q rows 0:1024 become the fillers
                    # of pair 3's qc1 passes
                    for qt_i in range(8):
                        fq.extend(fin_chain(qt_i))
        pump(1 << 30)

        # ---- tail: pipelined final projections for q rows 1024:2048,
        # interleaved with the last pass's normalization so the PE never
        # waits on the (long) reciprocal/broadcast chain ----
        class TailFin:
            def __init__(self, qt_i, use_pp):
                self.qt_i = qt_i
                self.use_pp = use_pp
                self.pool = p_ps
                self.halves = None
                self.whole = None

            def _alloc(self):
                qt_i = self.qt_i
                if self.use_pp:
                    t0 = p_pp.tile([128, SC], f32, tag="pp",
                                   name=f"fpp{qt_i}_0")
                    t1 = p_pp.tile([128, SC], f32, tag="pp",
                                   name=f"fpp{qt_i}_1")
                    self.halves = [t0[:], t1[:]]
                else:
                    tag = "ps" if self.pool is p_ps else "pv"
                    t = self.pool.tile([128, D], f32, tag=tag,
                                       name=f"fps{qt_i}")
                    self.halves = [t[:, 0:512], t[:, 512:1024]]
                    self.whole = t

            def mm(self, c_list):
                if self.halves is None:
                    self._alloc()
                for half in range(2):
                    for cc in c_list:
                        nc.tensor.matmul(
                            out=self.halves[half],
                            lhsT=ot_sb[cc][:, self.qt_i * 128:
                                           (self.qt_i + 1) * 128],
                            rhs=wo_sb[:, cc * D + half * 512:
                                      cc * D + (half + 1) * 512],
                            start=(cc == 0), stop=(cc == NCH - 1),
                        )

            def close(self):
                self.mm([3])
                stt = p_st.tile([128, D], f32, tag="st",
                                name=f"stt{self.qt_i}")
                if self.whole is not None:
                    nc.vector.tensor_copy(out=stt[:], in_=self.whole[:])
                else:
                    for half in range(2):
                        nc.vector.tensor_copy(
                            out=stt[:, half * 512:(half + 1) * 512],
                            in_=self.halves[half])
                _dma_rr[self.qt_i % 3].dma_start(
                    out=out[self.qt_i * 128:(self.qt_i + 1) * 128, :],
                    in_=stt[:])

        fins = {q: TailFin(q, use_pp=(q in (10, 13))) for q in range(8, KT)}
        fins[11].pool = p_pv
        fins[8].mm([0, 1, 2])
        normalize_hh(NCH - 1, NQC - 1, 0, ovts[0], last=True)
        fins[9].mm([0, 1, 2])
        fins[10].mm([0, 1, 2])
        fins[11].mm([0, 1, 2])
        for a, b in ((None, 8), (12, 9), (13, 10), (14, 11), (15, 12),
                     (None, 13), (None, 14), (None, 15)):
            if a is not None:
                fins[a].mm([0, 1, 2])
            fins[b].close()

    nc.compile()
    return nc


def get_program():
    if "nc" not in _CACHE:
        _CACHE["nc"] = _build_program()
    return _CACHE["nc"]


def _relayout_x(xt):
    # x^T [D, S] -> [128, NSC, DMT*SC]: [p, sc, a*SC+s] = xT[a*128+p, sc*SC+s]
    return np.ascontiguousarray(
        xt.reshape(DMT, 128, NSC, SC).transpose(1, 2, 0, 3)
    ).reshape(128, NSC, DMT * SC)


def make_in_maps(inputs):
    dt = _np_mm_dtype()
    q = np.asarray(inputs["query"], np.float32)
    k = np.asarray(inputs["key"], np.float32)
    v = np.asarray(inputs["value"], np.float32)
    Wq = np.asarray(inputs["Wq"], np.float32)
    Wk = np.asarray(inputs["Wk"], np.float32)
    Wv = np.asarray(inputs["Wv"], np.float32)
    Wo = np.asarray(inputs["Wo"], np.float32)
    bq = np.asarray(inputs["bq"], np.float32)
    bk = np.asarray(inputs["bk"], np.float32)
    in_maps = []
    xs_cache = {}
    for b in range(B):
        xs_cache[b] = {
            "xq": _relayout_x(q[b].T.astype(dt)),
            "xk": _relayout_x(k[b].T.astype(dt)),
            "xv": _relayout_x(v[b].T.astype(dt)),
        }
    for core in range(NCORES):
        b, g = core // 2, core % 2
        sl = slice(g * GD, (g + 1) * GD)
        # weights [D, GD] -> [128, DMT*GD]
        wq_a = np.ascontiguousarray(
            Wq[:, sl].astype(dt).reshape(DMT, 128, GD).transpose(1, 0, 2)
        ).reshape(128, DMT * GD)
        wk_a = np.ascontiguousarray(
            Wk[:, sl].astype(dt).reshape(DMT, 128, GD).transpose(1, 0, 2)
        ).reshape(128, DMT * GD)
        wv_a = np.ascontiguousarray(
            Wv[:, sl].astype(dt).reshape(DMT, 128, GD).transpose(1, 0, 2)
        ).reshape(128, DMT * GD)
        # wo [GD, D] -> [128, NCH*D]
        wo_a = np.ascontiguousarray(
            Wo[sl, :].astype(dt).reshape(NCH, 128, D).transpose(1, 0, 2)
        ).reshape(128, NCH * D)
        in_maps.append({
            "xq": xs_cache[b]["xq"],
            "xk": xs_cache[b]["xk"],
            "xv": xs_cache[b]["xv"],
            "wq": wq_a, "wk": wk_a, "wv": wv_a, "wo": wo_a,
            "bq": np.ascontiguousarray(bq[sl].reshape(NCH, 128).T),
            "bk": np.ascontiguousarray(bk[sl].reshape(NCH, 128).T),
        })
    return in_maps


def combine_outputs(results, inputs):
    Wo = np.asarray(inputs["Wo"], np.float32)
    bv = np.asarray(inputs["bv"], np.float32)
    bo = np.asarray(inputs["bo"], np.float32)
    out = np.empty((B, S, D), np.float32)
    for b in range(B):
        out[b] = results[2 * b]["out"] + results[2 * b + 1]["out"]
    out += bv @ Wo + bo
    return out


def kernel(**inputs):
    from concourse.bass_utils import run_bass_kernel_spmd
    nc = get_program()
    in_maps = make_in_maps(inputs)
    res = run_bass_kernel_spmd(nc, in_maps, list(range(NCORES)))
    return combine_outputs(res.results, inputs)
